# revision 7
# baseline (speedup 1.0000x reference)
"""Trainium2 Bass kernel for nn_BiLSTMModel (BiLSTM x2 + self-attention + maxpool + fc).

Sharding: data-parallel over batch B=64 across 8 cores (8 examples/core).
Each core processes 16 sequences (8 from x, 8 from y) fully independently:
embed-gather -> BiLSTM l0 -> BiLSTM l1 -> self-attention -> maxpool -> fc partial.
No collectives. Host concatenates per-core outputs and adds fc bias.

Layout convention on device: feature dims on partitions, (time, seq) on the free
axis ("transposed" layout). All matmuls are lhsT.T @ rhs with stationary weights.
"""

import numpy as np
import ml_dtypes

# Problem constants (hardcoded per the spec).
B, S, V, E, H = 64, 512, 10000, 256, 256
G = 4 * H  # 1024 gates
NCORES = 8
BL = B // NCORES          # 8 examples per core
NSEQ = 2 * BL             # 16 sequences per core (x then y)

_CACHE = {}


def _build_nc(T, nseq, chunk, debug=False):
    import concourse.mybir as mybir
    import concourse.tile as tile
    from concourse import bacc
    from concourse.masks import make_identity

    dt = mybir.dt
    f32, bf16, i16 = dt.float32, dt.bfloat16, dt.int16
    AF = mybir.ActivationFunctionType
    AX = mybir.AxisListType

    b = nseq
    nmac = T // chunk  # macro steps per layer

    nc = bacc.Bacc()

    emb = nc.declare_dram_parameter("embed", [V, E], bf16, isOutput=False)
    idx = nc.declare_dram_parameter("idx", [128, T], i16, isOutput=False)
    wihT0 = {d: nc.declare_dram_parameter(f"wihT_l0{d}", [128, 2, G], bf16, isOutput=False) for d in "fb"}
    whhT0 = {d: nc.declare_dram_parameter(f"whhT_l0{d}", [128, 2, G], bf16, isOutput=False) for d in "fb"}
    wihT1 = {d: nc.declare_dram_parameter(f"wihT_l1{d}", [128, 4, G], bf16, isOutput=False) for d in "fb"}
    whhT1 = {d: nc.declare_dram_parameter(f"whhT_l1{d}", [128, 2, G], bf16, isOutput=False) for d in "fb"}
    bias0 = {d: nc.declare_dram_parameter(f"bias_l0{d}", [128, 8], f32, isOutput=False) for d in "fb"}
    bias1 = {d: nc.declare_dram_parameter(f"bias_l1{d}", [128, 8], f32, isOutput=False) for d in "fb"}
    fcw = nc.declare_dram_parameter("fcw", [128, 8, 3], f32, isOutput=False)
    out_d = nc.declare_dram_parameter("out", [3, BL], f32, isOutput=True)
    dbg_d = nc.declare_dram_parameter("dbg", [128, 448], f32, isOutput=True) if debug else None

    def rec_step(psum_pool, scratch, whh_sb, pre_sb, s_loc, hbuf, col_prev, col_out, c):
        """One LSTM step in transposed layout: gates g^T = whh^T.T @ h^T + pre^T."""
        g = psum_pool.tile([128, 8, b], f32, name="g", tag="g")
        for j in range(8):
            for kk in range(2):
                nc.tensor.matmul(
                    g[:, j, :],
                    whh_sb[:, kk, j * 128:(j + 1) * 128],
                    hbuf[:, kk, col_prev, :],
                    start=(kk == 0),
                    stop=(kk == 1),
                )
        gsum = scratch.tile([128, 8, b], f32, name="gsum", tag="gsum")
        nc.vector.tensor_add(gsum[:], g[:], pre_sb[:, :, s_loc * b:(s_loc + 1) * b])
        sig_if = scratch.tile([128, 4, b], f32, name="sigif", tag="sigif")
        nc.scalar.activation(sig_if[:], gsum[:, 0:4, :], AF.Sigmoid)
        tanh_g = scratch.tile([128, 2, b], f32, name="tanhg", tag="tanhg")
        nc.scalar.activation(tanh_g[:], gsum[:, 4:6, :], AF.Tanh)
        sig_o = scratch.tile([128, 2, b], f32, name="sigo", tag="sigo")
        nc.scalar.activation(sig_o[:], gsum[:, 6:8, :], AF.Sigmoid)
        t1 = scratch.tile([128, 2, b], f32, name="t1", tag="t1")
        nc.vector.tensor_mul(t1[:], sig_if[:, 2:4, :], c[:])
        t2 = scratch.tile([128, 2, b], f32, name="t2", tag="t2")
        nc.vector.tensor_mul(t2[:], sig_if[:, 0:2, :], tanh_g[:])
        nc.vector.tensor_add(c[:], t1[:], t2[:])
        tc_t = scratch.tile([128, 2, b], f32, name="tct", tag="tct")
        nc.scalar.activation(tc_t[:], c[:], AF.Tanh)
        nc.vector.tensor_mul(hbuf[:, :, col_out, :], sig_o[:], tc_t[:])

    def evac(j, psum_ap, dst_ap, bias_sb):
        """PSUM -> SBUF bf16 with per-partition bias fold; alternate engines."""
        if j % 2 == 0:
            nc.vector.tensor_scalar_add(dst_ap, psum_ap, bias_sb[:, j:j + 1])
        else:
            nc.scalar.add(dst_ap, psum_ap, bias_sb[:, j:j + 1])

    with tile.TileContext(nc) as tc:
        with tc.tile_pool(name="persist", bufs=1) as P:
            ident = P.tile([128, 128], bf16, name="ident", tag="ident")
            make_identity(nc, ident[:])
            idx_sb = P.tile([128, T], i16, name="idx", tag="idx")
            nc.sync.dma_start(idx_sb[:], idx[:])
            fcw_sb = P.tile([128, 8, 3], f32, name="fcw", tag="fcw")
            nc.sync.dma_start(fcw_sb[:], fcw[:])
            z_all = P.tile([128, 64], f32, name="zall", tag="zall")  # col = src*8 + example
            dbg_sb = P.tile([128, 448], f32, name="dbgsb", tag="dbgsb") if debug else None
            h1 = {}
            for d in "fb":
                h1[d] = P.tile([128, 2, T + 1, b], bf16, name=f"h1{d}", tag=f"h1{d}")

            with tc.tile_pool(name="mid", bufs=1) as M:
                h0 = {}
                for d in "fb":
                    h0[d] = M.tile([128, 2, T + 1, b], bf16, name=f"h0{d}", tag=f"h0{d}")

                # ---------------- layer 0 ----------------
                with tc.tile_pool(name="ph0", bufs=1) as P0, \
                     tc.tile_pool(name="ebuf", bufs=2) as EB, \
                     tc.tile_pool(name="prebuf", bufs=2) as PB, \
                     tc.tile_pool(name="scr", bufs=4) as SC, \
                     tc.tile_pool(name="pg", bufs=4, space="PSUM") as PG, \
                     tc.tile_pool(name="pp", bufs=4, space="PSUM") as PP:
                    wih_sb = {d: P0.tile([128, 2, G], bf16, name=f"wih{d}", tag=f"wih{d}") for d in "fb"}
                    whh_sb = {d: P0.tile([128, 2, G], bf16, name=f"whh{d}", tag=f"whh{d}") for d in "fb"}
                    bias_sb = {d: P0.tile([128, 8], f32, name=f"bias{d}", tag=f"bias{d}") for d in "fb"}
                    for d in "fb":
                        nc.sync.dma_start(wih_sb[d][:], wihT0[d][:])
                        nc.sync.dma_start(whh_sb[d][:], whhT0[d][:])
                        nc.sync.dma_start(bias_sb[d][:], bias0[d][:])
                    c0 = {}
                    for d in "fb":
                        c0[d] = P0.tile([128, 2, b], f32, name=f"c0{d}", tag=f"c0{d}")
                        nc.vector.memset(c0[d][:], 0.0)
                        nc.vector.memset(h0[d][:, :, T if d == "b" else 0, :], 0.0)

                    for m in range(nmac):
                        t0 = {"f": m * chunk, "b": T - (m + 1) * chunk}
                        et = {}
                        pre = {}
                        for d in "fb":
                            et[d] = EB.tile([128, 2, chunk * b], bf16, name=f"et{d}", tag=f"et{d}")
                            nc.gpsimd.dma_gather(
                                et[d][:], emb[:], idx_sb[:, t0[d]:t0[d] + chunk],
                                chunk * 16, chunk * 16, E, transpose=True,
                            )
                            if debug and m == 0 and d == "f":
                                nc.vector.tensor_copy(dbg_sb[:, 0:64], et[d][:, 0, 0:64])
                            pre[d] = PB.tile([128, 8, chunk * b], bf16, name=f"pre{d}", tag=f"pre{d}")
                            for j in range(8):
                                ps = PP.tile([128, chunk * b], f32, name="ps", tag="ps")
                                for kk in range(2):
                                    nc.tensor.matmul(
                                        ps[:], wih_sb[d][:, kk, j * 128:(j + 1) * 128],
                                        et[d][:, kk, :], start=(kk == 0), stop=(kk == 1),
                                    )
                                evac(j, ps[:], pre[d][:, j, :], bias_sb[d])
                        if debug and m == 0:
                            nc.vector.tensor_copy(dbg_sb[:, 64:128], pre["f"][:, 0, 0:64])
                        for s in range(chunk):
                            tf = t0["f"] + s
                            tb = T - 1 - (m * chunk + s)
                            sb_ = chunk - 1 - s  # local index of tb within its chunk
                            rec_step(PG, SC, whh_sb["f"], pre["f"], s, h0["f"], tf, tf + 1, c0["f"])
                            rec_step(PG, SC, whh_sb["b"], pre["b"], sb_, h0["b"], tb + 1, tb, c0["b"])
                    if debug:
                        nc.vector.tensor_copy(dbg_sb[:, 128:256], h0["f"][:, 0, 1:9, :])

                # ---------------- layer 1 ----------------
                with tc.tile_pool(name="ph1", bufs=1) as P1, \
                     tc.tile_pool(name="prebuf1", bufs=2) as PB1, \
                     tc.tile_pool(name="scr1", bufs=4) as SC1, \
                     tc.tile_pool(name="pg1", bufs=4, space="PSUM") as PG1, \
                     tc.tile_pool(name="pp1", bufs=4, space="PSUM") as PP1:
                    wih1_sb = {d: P1.tile([128, 4, G], bf16, name=f"wih1{d}", tag=f"wih1{d}") for d in "fb"}
                    whh1_sb = {d: P1.tile([128, 2, G], bf16, name=f"whh1{d}", tag=f"whh1{d}") for d in "fb"}
                    bias1_sb = {d: P1.tile([128, 8], f32, name=f"bias1{d}", tag=f"bias1{d}") for d in "fb"}
                    for d in "fb":
                        nc.sync.dma_start(wih1_sb[d][:], wihT1[d][:])
                        nc.sync.dma_start(whh1_sb[d][:], whhT1[d][:])
                        nc.sync.dma_start(bias1_sb[d][:], bias1[d][:])
                    c1 = {}
                    for d in "fb":
                        c1[d] = P1.tile([128, 2, b], f32, name=f"c1{d}", tag=f"c1{d}")
                        nc.vector.memset(c1[d][:], 0.0)
                        nc.vector.memset(h1[d][:, :, T if d == "b" else 0, :], 0.0)

                    for m in range(nmac):
                        t0 = {"f": m * chunk, "b": T - (m + 1) * chunk}
                        pre = {}
                        for d in "fb":
                            pre[d] = PB1.tile([128, 8, chunk * b], bf16, name=f"pre1{d}", tag=f"pre1{d}")
                            for j in range(8):
                                ps = PP1.tile([128, chunk * b], f32, name="ps1", tag="ps1")
                                # K = 512: kk 0,1 from l0-fwd (cols shifted +1), kk 2,3 from l0-bwd
                                for kk in range(4):
                                    src = h0["f"] if kk < 2 else h0["b"]
                                    base = 1 if kk < 2 else 0
                                    nc.tensor.matmul(
                                        ps[:], wih1_sb[d][:, kk, j * 128:(j + 1) * 128],
                                        src[:, kk % 2, base + t0[d]:base + t0[d] + chunk, :],
                                        start=(kk == 0), stop=(kk == 3),
                                    )
                                evac(j, ps[:], pre[d][:, j, :], bias1_sb[d])
                        for s in range(chunk):
                            tf = t0["f"] + s
                            tb = T - 1 - (m * chunk + s)
                            sb_ = chunk - 1 - s
                            rec_step(PG1, SC1, whh1_sb["f"], pre["f"], s, h1["f"], tf, tf + 1, c1["f"])
                            rec_step(PG1, SC1, whh1_sb["b"], pre["b"], sb_, h1["b"], tb + 1, tb, c1["b"])
                    if debug:
                        nc.vector.tensor_copy(dbg_sb[:, 256:384], h1["f"][:, 0, 1:9, :])

            # ---------------- attention + maxpool ----------------
            TT = T // 128  # number of 128-tiles along time
            dtiles = [("f", 0), ("f", 1), ("b", 0), ("b", 1)]  # concat order of d=512
            with tc.tile_pool(name="attn", bufs=2) as A, \
                 tc.tile_pool(name="attn1", bufs=2) as A1, \
                 tc.tile_pool(name="ps_s", bufs=2, space="PSUM") as PS, \
                 tc.tile_pool(name="ps_tr", bufs=2, space="PSUM") as PTR, \
                 tc.tile_pool(name="ps_o", bufs=2, space="PSUM") as PO:
                for ex in range(b):
                    # h_ex[p, tt, d] = h[tt*128+p, d]  (transposed copy of h^T)
                    h_ex = A.tile([128, TT, 512], bf16, name="hex", tag="hex")
                    for tt in range(TT):
                        for kki, (d, kk) in enumerate(dtiles):
                            base = 1 if d == "f" else 0
                            ptr = PTR.tile([128, 128], bf16, name="ptr", tag="ptr")
                            nc.tensor.transpose(
                                ptr[:],
                                h1[d][:, kk, base + tt * 128:base + (tt + 1) * 128, ex],
                                ident[:],
                            )
                            nc.vector.tensor_copy(h_ex[:, tt, kki * 128:(kki + 1) * 128], ptr[:])
                    # scores + softmax + a^T
                    aT = A.tile([128, TT, T], bf16, name="aT", tag="aT")
                    for t1t in range(TT):
                        s_ps = PS.tile([128, T], f32, name="sps", tag="sps")
                        for kki, (d, kk) in enumerate(dtiles):
                            base = 1 if d == "f" else 0
                            nc.tensor.matmul(
                                s_ps[:],
                                h1[d][:, kk, base + t1t * 128:base + (t1t + 1) * 128, ex],
                                h1[d][:, kk, base:base + T, ex],
                                start=(kki == 0), stop=(kki == 3),
                            )
                        mx = A1.tile([128, 1], f32, name="mx", tag="mx")
                        nc.vector.reduce_max(mx[:], s_ps[:], axis=AX.X)
                        nmx = A1.tile([128, 1], f32, name="nmx", tag="nmx")
                        nc.vector.tensor_scalar_mul(nmx[:], mx[:], -1.0)
                        expS = A1.tile([128, T], bf16, name="expS", tag="expS")
                        sume = A1.tile([128, 1], f32, name="sume", tag="sume")
                        nc.scalar.activation(expS[:], s_ps[:], AF.Exp, bias=nmx[:], scale=1.0, accum_out=sume[:])
                        rcp = A1.tile([128, 1], f32, name="rcp", tag="rcp")
                        nc.vector.reciprocal(rcp[:], sume[:])
                        a_t = A1.tile([128, T], bf16, name="a_t", tag="a_t")
                        nc.vector.tensor_scalar_mul(a_t[:], expS[:], rcp[:])
                        for t2t in range(TT):
                            ptr = PTR.tile([128, 128], bf16, name="ptr", tag="ptr")
                            nc.tensor.transpose(ptr[:], a_t[:, t2t * 128:(t2t + 1) * 128], ident[:])
                            nc.vector.tensor_copy(aT[:, t2t, t1t * 128:(t1t + 1) * 128], ptr[:])
                    # o^T = h^T @ a^T ; maxpool over t1 (free dim)
                    enc = ex // BL  # 0 = x-encoding, 1 = y-encoding
                    e_i = ex % BL
                    for dkk in range(4):
                        o_ps = PO.tile([128, T], f32, name="ops", tag="ops")
                        for t2t in range(TT):
                            nc.tensor.matmul(
                                o_ps[:],
                                h_ex[:, t2t, dkk * 128:(dkk + 1) * 128],
                                aT[:, t2t, :],
                                start=(t2t == 0), stop=(t2t == TT - 1),
                            )
                        k = (dkk + 4 * enc) * 8 + e_i
                        nc.vector.reduce_max(z_all[:, k:k + 1], o_ps[:], axis=AX.X)

                # ---------------- fc ----------------
                fc_ps = PS.tile([3, BL], f32, name="fcps", tag="fcps", bufs=1)
                for src in range(8):
                    nc.tensor.matmul(
                        fc_ps[:], fcw_sb[:, src, :], z_all[:, src * 8:src * 8 + BL],
                        start=(src == 0), stop=(src == 7),
                    )
                out_sb = A1.tile([3, BL], f32, name="outsb", tag="outsb")
                nc.vector.tensor_copy(out_sb[:], fc_ps[:])
                nc.sync.dma_start(out_d[:], out_sb[:])
                if debug:
                    nc.vector.tensor_copy(dbg_sb[:, 384:448], z_all[:])
                    nc.sync.dma_start(dbg_d[:], dbg_sb[:])

    nc.compile()
    return nc


def _prep_shared(inputs):
    """Host-side weight rearrangement (shared across cores)."""
    bf16 = ml_dtypes.bfloat16

    def wT(w, kt):  # [G, K] -> [128, kt, G] with [p, kk, g] = w[g, kk*128+p]
        return np.ascontiguousarray(w.T.reshape(kt, 128, w.shape[0]).transpose(1, 0, 2)).astype(bf16)

    d = {"embed": np.ascontiguousarray(inputs["embed"]).astype(bf16)}
    for L, kt in (("0", 2), ("1", 4)):
        for dd in "fb":
            d[f"wihT_l{L}{dd}"] = wT(np.asarray(inputs[f"wih_l{L}{dd}"]), kt)
            d[f"whhT_l{L}{dd}"] = wT(np.asarray(inputs[f"whh_l{L}{dd}"]), 2)
            d[f"bias_l{L}{dd}"] = np.ascontiguousarray(
                np.asarray(inputs[f"b_l{L}{dd}"]).reshape(8, 128).T).astype(np.float32)
    fc_w = np.asarray(inputs["fc_w"])  # [3, 1024]
    d["fcw"] = np.ascontiguousarray(fc_w.T.reshape(8, 128, 3).transpose(1, 0, 2)).astype(np.float32)
    return d


def _per_core_inputs(inputs, shared):
    x = np.asarray(inputs["x"])
    y = np.asarray(inputs["y"])
    maps = []
    for i in range(NCORES):
        idx16 = np.concatenate(
            [x[i * BL:(i + 1) * BL], y[i * BL:(i + 1) * BL]], 0).astype(np.int16)
        # idxs are read 16-partitions-per-GPSIMD-core, replicated across 8 cores
        idx = np.tile(idx16, (8, 1))
        m = dict(shared)
        m["idx"] = idx
        maps.append(m)
    return maps


def _get_exec():
    key = "main"
    if key not in _CACHE:
        nc = _build_nc(S, NSEQ, 32)
        _CACHE[key] = nc
    return _CACHE[key]


def kernel(**inputs) -> np.ndarray:
    from concourse.bass_utils import run_bass_kernel_spmd

    nc = _get_exec()
    shared = _prep_shared(inputs)
    in_maps = _per_core_inputs(inputs, shared)
    res = run_bass_kernel_spmd(nc, in_maps, core_ids=list(range(NCORES)))
    fc_b = np.asarray(inputs["fc_b"]).astype(np.float32)
    out = np.zeros((B, 3), np.float32)
    for i in range(NCORES):
        out[i * BL:(i + 1) * BL, :] = res.results[i]["out"].T + fc_b[None, :]
    return out


# revision 14
# speedup vs baseline: 25.0725x; 25.0725x over previous
"""Trainium2 Bass kernel for nn_BiLSTMModel (BiLSTM x2 + self-attention + maxpool + fc).

Sharding: data-parallel over batch B=64 across 8 cores (8 examples/core).
Each core processes 16 sequences (8 from x, 8 from y) fully independently:
embed-gather -> BiLSTM l0 -> BiLSTM l1 -> self-attention -> maxpool -> fc partial.
No collectives. Host concatenates per-core outputs and adds fc bias.

Layout convention on device: feature dims on partitions, (time, seq) on the free
axis ("transposed" layout). All matmuls are lhsT.T @ rhs with stationary weights.
"""

import numpy as np
import ml_dtypes

# Problem constants (hardcoded per the spec).
B, S, V, E, H = 64, 512, 10000, 256, 256
G = 4 * H  # 1024 gates
NCORES = 8
BL = B // NCORES          # 8 examples per core
NSEQ = 2 * BL             # 16 sequences per core (x then y)

_CACHE = {}


def _build_nc(T, nseq, chunk, debug=False):
    import concourse.mybir as mybir
    import concourse.tile as tile
    from concourse import bacc
    from concourse.masks import make_identity

    dt = mybir.dt
    f32, bf16, i16 = dt.float32, dt.bfloat16, dt.int16
    AF = mybir.ActivationFunctionType
    AX = mybir.AxisListType

    b = nseq
    nmac = T // chunk  # macro steps per layer

    nc = bacc.Bacc()

    emb = nc.declare_dram_parameter("embed", [V, E], bf16, isOutput=False)
    idx = nc.declare_dram_parameter("idx", [128, T], i16, isOutput=False)
    wihT0 = {d: nc.declare_dram_parameter(f"wihT_l0{d}", [128, 2, G], bf16, isOutput=False) for d in "fb"}
    whhT0 = {d: nc.declare_dram_parameter(f"whhT_l0{d}", [128, 2, G], bf16, isOutput=False) for d in "fb"}
    wihT1 = {d: nc.declare_dram_parameter(f"wihT_l1{d}", [128, 4, G], bf16, isOutput=False) for d in "fb"}
    whhT1 = {d: nc.declare_dram_parameter(f"whhT_l1{d}", [128, 2, G], bf16, isOutput=False) for d in "fb"}
    bias0 = {d: nc.declare_dram_parameter(f"bias_l0{d}", [128, 8], f32, isOutput=False) for d in "fb"}
    bias1 = {d: nc.declare_dram_parameter(f"bias_l1{d}", [128, 8], f32, isOutput=False) for d in "fb"}
    fcw = nc.declare_dram_parameter("fcw", [128, 8, 3], f32, isOutput=False)
    out_d = nc.declare_dram_parameter("out", [3, BL], f32, isOutput=True)
    dbg_d = nc.declare_dram_parameter("dbg", [128, 448], f32, isOutput=True) if debug else None

    def rec_pair(psum_pool, scratch, whh_sb, pre_sb, s_loc, hbuf, col_prev, col_out, c):
        """One LSTM time step for both chains, decoupled (independent dep chains).

        Gate order host-permuted to [i, f, o, g]; g-gate weight rows are
        host-scaled by 2 so tanh(u) = 2*sigmoid(2u) - 1 needs one sigmoid
        over all 8 gate tiles.
        """
        import os as _os
        nkk = 1 if _os.environ.get("REC_HALF_MM") else 2
        for d in "fb":
            g = psum_pool.tile([128, 8, b], f32, name=f"g{d}", tag=f"g{d}", bufs=3)
            for j in range(8):
                for kk in range(nkk):
                    nc.tensor.matmul(
                        g[:, j, :],
                        whh_sb[d][:, kk, j * 128:(j + 1) * 128],
                        hbuf[d][:, kk, col_prev[d], :],
                        start=(kk == 0),
                        stop=(kk == nkk - 1),
                    )
            gsum = scratch.tile([128, 8, b], f32, name=f"gsum{d}", tag=f"gsum{d}")
            nc.vector.tensor_add(gsum[:], g[:], pre_sb[d][:, :, s_loc[d] * b:(s_loc[d] + 1) * b])
            sig = scratch.tile([128, 8, b], bf16, name=f"sig{d}", tag=f"sig{d}")
            nc.scalar.activation(sig[:], gsum[:], AF.Sigmoid)
            tg = scratch.tile([128, 2, b], bf16, name=f"tg{d}", tag=f"tg{d}")
            nc.vector.tensor_scalar(tg[:], sig[:, 6:8, :], 2.0, -1.0,
                                    op0=mybir.AluOpType.mult, op1=mybir.AluOpType.add)
            t1 = scratch.tile([128, 2, b], f32, name=f"t1{d}", tag=f"t1{d}")
            nc.vector.tensor_mul(t1[:], sig[:, 2:4, :], c[d][:])
            t2 = scratch.tile([128, 2, b], f32, name=f"t2{d}", tag=f"t2{d}")
            nc.vector.tensor_mul(t2[:], sig[:, 0:2, :], tg[:])
            nc.vector.tensor_add(c[d][:], t1[:], t2[:])
            tc_t = scratch.tile([128, 2, b], bf16, name=f"tct{d}", tag=f"tct{d}")
            nc.scalar.activation(tc_t[:], c[d][:], AF.Tanh)
            nc.vector.tensor_mul(hbuf[d][:, :, col_out[d], :], sig[:, 4:6, :], tc_t[:])

    def evac(j, psum_ap, dst_ap, bias_sb):
        """PSUM -> SBUF bf16 with per-partition bias fold; alternate engines."""
        if j % 2 == 0:
            nc.vector.tensor_scalar_add(dst_ap, psum_ap, bias_sb[:, j:j + 1])
        else:
            nc.scalar.add(dst_ap, psum_ap, bias_sb[:, j:j + 1])

    with tile.TileContext(nc) as tc:
        with tc.tile_pool(name="persist", bufs=1) as P:
            ident = P.tile([128, 128], bf16, name="ident", tag="ident")
            make_identity(nc, ident[:])
            idx_sb = P.tile([128, T], i16, name="idx", tag="idx")
            nc.sync.dma_start(idx_sb[:], idx[:])
            fcw_sb = P.tile([128, 8, 3], f32, name="fcw", tag="fcw")
            nc.sync.dma_start(fcw_sb[:], fcw[:])
            z_all = P.tile([128, 64], f32, name="zall", tag="zall")  # col = src*8 + example
            dbg_sb = P.tile([128, 448], f32, name="dbgsb", tag="dbgsb") if debug else None
            h1 = {}
            for d in "fb":
                h1[d] = P.tile([128, 2, T + 1, b], bf16, name=f"h1{d}", tag=f"h1{d}")

            with tc.tile_pool(name="mid", bufs=1) as M:
                h0 = {}
                for d in "fb":
                    h0[d] = M.tile([128, 2, T + 1, b], bf16, name=f"h0{d}", tag=f"h0{d}")

                # ---------------- layer 0 ----------------
                with tc.tile_pool(name="ph0", bufs=1) as P0, \
                     tc.tile_pool(name="ebuf", bufs=2) as EB, \
                     tc.tile_pool(name="prebuf", bufs=2) as PB, \
                     tc.tile_pool(name="scr", bufs=4) as SC, \
                     tc.tile_pool(name="pg", bufs=4, space="PSUM") as PG, \
                     tc.tile_pool(name="pp", bufs=2, space="PSUM") as PP:
                    wih_sb = {d: P0.tile([128, 2, G], bf16, name=f"wih{d}", tag=f"wih{d}") for d in "fb"}
                    whh_sb = {d: P0.tile([128, 2, G], bf16, name=f"whh{d}", tag=f"whh{d}") for d in "fb"}
                    bias_sb = {d: P0.tile([128, 8], f32, name=f"bias{d}", tag=f"bias{d}") for d in "fb"}
                    for d in "fb":
                        nc.sync.dma_start(wih_sb[d][:], wihT0[d][:])
                        nc.sync.dma_start(whh_sb[d][:], whhT0[d][:])
                        nc.sync.dma_start(bias_sb[d][:], bias0[d][:])
                    c0 = {}
                    for d in "fb":
                        c0[d] = P0.tile([128, 2, b], f32, name=f"c0{d}", tag=f"c0{d}")
                        nc.vector.memset(c0[d][:], 0.0)
                        nc.vector.memset(h0[d][:, :, T if d == "b" else 0, :], 0.0)

                    for m in range(nmac):
                        t0 = {"f": m * chunk, "b": T - (m + 1) * chunk}
                        et = {}
                        pre = {}
                        for d in "fb":
                            et[d] = EB.tile([128, 2, chunk * b], bf16, name=f"et{d}", tag=f"et{d}")
                            nc.gpsimd.dma_gather(
                                et[d][:], emb[:], idx_sb[:, t0[d]:t0[d] + chunk],
                                chunk * 16, chunk * 16, E, transpose=True,
                            )
                            if debug and m == 0 and d == "f":
                                nc.vector.tensor_copy(dbg_sb[:, 0:64], et[d][:, 0, 0:64])
                            pre[d] = PB.tile([128, 8, chunk * b], bf16, name=f"pre{d}", tag=f"pre{d}")
                            for j in range(8):
                                ps = PP.tile([128, chunk * b], f32, name="ps", tag="ps")
                                for kk in range(2):
                                    nc.tensor.matmul(
                                        ps[:], wih_sb[d][:, kk, j * 128:(j + 1) * 128],
                                        et[d][:, kk, :], start=(kk == 0), stop=(kk == 1),
                                    )
                                evac(j, ps[:], pre[d][:, j, :], bias_sb[d])
                        if debug and m == 0:
                            nc.vector.tensor_copy(dbg_sb[:, 64:128], pre["f"][:, 0, 0:64])
                        for s in range(chunk):
                            tf = t0["f"] + s
                            tb = T - 1 - (m * chunk + s)
                            sb_ = chunk - 1 - s  # local index of tb within its chunk
                            rec_pair(PG, SC, whh_sb, pre, {"f": s, "b": sb_}, h0,
                                     {"f": tf, "b": tb + 1}, {"f": tf + 1, "b": tb}, c0)
                    if debug:
                        nc.vector.tensor_copy(dbg_sb[:, 128:256], h0["f"][:, 0, 1:9, :])

                # ---------------- layer 1 ----------------
                with tc.tile_pool(name="ph1", bufs=1) as P1, \
                     tc.tile_pool(name="prebuf1", bufs=2) as PB1, \
                     tc.tile_pool(name="scr1", bufs=4) as SC1, \
                     tc.tile_pool(name="pg1", bufs=4, space="PSUM") as PG1, \
                     tc.tile_pool(name="pp1", bufs=2, space="PSUM") as PP1:
                    wih1_sb = {d: P1.tile([128, 4, G], bf16, name=f"wih1{d}", tag=f"wih1{d}") for d in "fb"}
                    whh1_sb = {d: P1.tile([128, 2, G], bf16, name=f"whh1{d}", tag=f"whh1{d}") for d in "fb"}
                    bias1_sb = {d: P1.tile([128, 8], f32, name=f"bias1{d}", tag=f"bias1{d}") for d in "fb"}
                    for d in "fb":
                        nc.sync.dma_start(wih1_sb[d][:], wihT1[d][:])
                        nc.sync.dma_start(whh1_sb[d][:], whhT1[d][:])
                        nc.sync.dma_start(bias1_sb[d][:], bias1[d][:])
                    c1 = {}
                    for d in "fb":
                        c1[d] = P1.tile([128, 2, b], f32, name=f"c1{d}", tag=f"c1{d}")
                        nc.vector.memset(c1[d][:], 0.0)
                        nc.vector.memset(h1[d][:, :, T if d == "b" else 0, :], 0.0)

                    for m in range(nmac):
                        t0 = {"f": m * chunk, "b": T - (m + 1) * chunk}
                        pre = {}
                        for d in "fb":
                            pre[d] = PB1.tile([128, 8, chunk * b], bf16, name=f"pre1{d}", tag=f"pre1{d}")
                            for j in range(8):
                                ps = PP1.tile([128, chunk * b], f32, name="ps1", tag="ps1")
                                # K = 512: kk 0,1 from l0-fwd (cols shifted +1), kk 2,3 from l0-bwd
                                for kk in range(4):
                                    src = h0["f"] if kk < 2 else h0["b"]
                                    base = 1 if kk < 2 else 0
                                    nc.tensor.matmul(
                                        ps[:], wih1_sb[d][:, kk, j * 128:(j + 1) * 128],
                                        src[:, kk % 2, base + t0[d]:base + t0[d] + chunk, :],
                                        start=(kk == 0), stop=(kk == 3),
                                    )
                                evac(j, ps[:], pre[d][:, j, :], bias1_sb[d])
                        for s in range(chunk):
                            tf = t0["f"] + s
                            tb = T - 1 - (m * chunk + s)
                            sb_ = chunk - 1 - s
                            rec_pair(PG1, SC1, whh1_sb, pre, {"f": s, "b": sb_}, h1,
                                     {"f": tf, "b": tb + 1}, {"f": tf + 1, "b": tb}, c1)
                    if debug:
                        nc.vector.tensor_copy(dbg_sb[:, 256:384], h1["f"][:, 0, 1:9, :])

            # ---------------- attention + maxpool ----------------
            TT = T // 128  # number of 128-tiles along time
            dtiles = [("f", 0), ("f", 1), ("b", 0), ("b", 1)]  # concat order of d=512
            with tc.tile_pool(name="attn", bufs=2) as A, \
                 tc.tile_pool(name="attn1", bufs=2) as A1, \
                 tc.tile_pool(name="ps_s", bufs=2, space="PSUM") as PS, \
                 tc.tile_pool(name="ps_tr", bufs=2, space="PSUM") as PTR, \
                 tc.tile_pool(name="ps_o", bufs=2, space="PSUM") as PO:
                for ex in range(b):
                    # h_ex[p, tt, d] = h[tt*128+p, d]  (transposed copy of h^T)
                    h_ex = A.tile([128, TT, 512], bf16, name="hex", tag="hex")
                    for tt in range(TT):
                        for kki, (d, kk) in enumerate(dtiles):
                            base = 1 if d == "f" else 0
                            ptr = PTR.tile([128, 128], bf16, name="ptr", tag="ptr")
                            nc.tensor.transpose(
                                ptr[:],
                                h1[d][:, kk, base + tt * 128:base + (tt + 1) * 128, ex],
                                ident[:],
                            )
                            nc.vector.tensor_copy(h_ex[:, tt, kki * 128:(kki + 1) * 128], ptr[:])
                    # scores + softmax + a^T
                    aT = A.tile([128, TT, T], bf16, name="aT", tag="aT")
                    for t1t in range(TT):
                        s_ps = PS.tile([128, T], f32, name="sps", tag="sps")
                        for kki, (d, kk) in enumerate(dtiles):
                            base = 1 if d == "f" else 0
                            nc.tensor.matmul(
                                s_ps[:],
                                h1[d][:, kk, base + t1t * 128:base + (t1t + 1) * 128, ex],
                                h1[d][:, kk, base:base + T, ex],
                                start=(kki == 0), stop=(kki == 3),
                            )
                        mx = A1.tile([128, 1], f32, name="mx", tag="mx")
                        nc.vector.reduce_max(mx[:], s_ps[:], axis=AX.X)
                        nmx = A1.tile([128, 1], f32, name="nmx", tag="nmx")
                        nc.vector.tensor_scalar_mul(nmx[:], mx[:], -1.0)
                        expS = A1.tile([128, T], bf16, name="expS", tag="expS")
                        sume = A1.tile([128, 1], f32, name="sume", tag="sume")
                        nc.scalar.activation(expS[:], s_ps[:], AF.Exp, bias=nmx[:], scale=1.0, accum_out=sume[:])
                        rcp = A1.tile([128, 1], f32, name="rcp", tag="rcp")
                        nc.vector.reciprocal(rcp[:], sume[:])
                        a_t = A1.tile([128, T], bf16, name="a_t", tag="a_t")
                        nc.vector.tensor_scalar_mul(a_t[:], expS[:], rcp[:])
                        for t2t in range(TT):
                            ptr = PTR.tile([128, 128], bf16, name="ptr", tag="ptr")
                            nc.tensor.transpose(ptr[:], a_t[:, t2t * 128:(t2t + 1) * 128], ident[:])
                            nc.vector.tensor_copy(aT[:, t2t, t1t * 128:(t1t + 1) * 128], ptr[:])
                    # o^T = h^T @ a^T ; maxpool over t1 (free dim)
                    enc = ex // BL  # 0 = x-encoding, 1 = y-encoding
                    e_i = ex % BL
                    for dkk in range(4):
                        o_ps = PO.tile([128, T], f32, name="ops", tag="ops")
                        for t2t in range(TT):
                            nc.tensor.matmul(
                                o_ps[:],
                                h_ex[:, t2t, dkk * 128:(dkk + 1) * 128],
                                aT[:, t2t, :],
                                start=(t2t == 0), stop=(t2t == TT - 1),
                            )
                        k = (dkk + 4 * enc) * 8 + e_i
                        nc.vector.reduce_max(z_all[:, k:k + 1], o_ps[:], axis=AX.X)

                # ---------------- fc ----------------
                fc_ps = PS.tile([3, BL], f32, name="fcps", tag="fcps", bufs=1)
                for src in range(8):
                    nc.tensor.matmul(
                        fc_ps[:], fcw_sb[:, src, :], z_all[:, src * 8:src * 8 + BL],
                        start=(src == 0), stop=(src == 7),
                    )
                out_sb = A1.tile([3, BL], f32, name="outsb", tag="outsb")
                nc.vector.tensor_copy(out_sb[:], fc_ps[:])
                nc.sync.dma_start(out_d[:], out_sb[:])
                if debug:
                    nc.vector.tensor_copy(dbg_sb[:, 384:448], z_all[:])
                    nc.sync.dma_start(dbg_d[:], dbg_sb[:])

    nc.compile()
    return nc


def _prep_shared(inputs):
    """Host-side weight rearrangement (shared across cores)."""
    bf16 = ml_dtypes.bfloat16

    def gperm(w):  # reorder gate rows [i,f,g,o] -> [i,f,o,g]; scale g rows by 2
        return np.concatenate([w[0:512], w[768:1024], 2.0 * w[512:768]], 0)

    def wT(w, kt):  # [G, K] -> [128, kt, G] with [p, kk, g] = w[g, kk*128+p]
        w = gperm(w)
        return np.ascontiguousarray(w.T.reshape(kt, 128, w.shape[0]).transpose(1, 0, 2)).astype(bf16)

    d = {"embed": np.ascontiguousarray(inputs["embed"]).astype(bf16)}
    for L, kt in (("0", 2), ("1", 4)):
        for dd in "fb":
            d[f"wihT_l{L}{dd}"] = wT(np.asarray(inputs[f"wih_l{L}{dd}"]), kt)
            d[f"whhT_l{L}{dd}"] = wT(np.asarray(inputs[f"whh_l{L}{dd}"]), 2)
            d[f"bias_l{L}{dd}"] = np.ascontiguousarray(
                gperm(np.asarray(inputs[f"b_l{L}{dd}"])).reshape(8, 128).T).astype(np.float32)
    fc_w = np.asarray(inputs["fc_w"])  # [3, 1024]
    d["fcw"] = np.ascontiguousarray(fc_w.T.reshape(8, 128, 3).transpose(1, 0, 2)).astype(np.float32)
    return d


def _per_core_inputs(inputs, shared):
    x = np.asarray(inputs["x"])
    y = np.asarray(inputs["y"])
    maps = []
    for i in range(NCORES):
        idx16 = np.concatenate(
            [x[i * BL:(i + 1) * BL], y[i * BL:(i + 1) * BL]], 0).astype(np.int16)
        # idxs are read 16-partitions-per-GPSIMD-core, replicated across 8 cores
        idx = np.tile(idx16, (8, 1))
        m = dict(shared)
        m["idx"] = idx
        maps.append(m)
    return maps


def _get_exec():
    key = "main"
    if key not in _CACHE:
        nc = _build_nc(S, NSEQ, 32)
        _CACHE[key] = nc
    return _CACHE[key]


def kernel(**inputs) -> np.ndarray:
    from concourse.bass_utils import run_bass_kernel_spmd

    nc = _get_exec()
    shared = _prep_shared(inputs)
    in_maps = _per_core_inputs(inputs, shared)
    res = run_bass_kernel_spmd(nc, in_maps, core_ids=list(range(NCORES)))
    fc_b = np.asarray(inputs["fc_b"]).astype(np.float32)
    out = np.zeros((B, 3), np.float32)
    for i in range(NCORES):
        out[i * BL:(i + 1) * BL, :] = res.results[i]["out"].T + fc_b[None, :]
    return out
